# revision 6
# baseline (speedup 1.0000x reference)
"""DGL-style 2-layer GAT on 8 TRN2 NeuronCores (Bass/Tile).

Sharding (per sharding_hint): dst nodes + incident edges partitioned
across 8 cores; weights replicated; src features shared via AllGather.

Node ownership: core r owns node rows A_r=[6250r,6250(r+1)) and
B_r=[50000+6250r, 50000+6250(r+1)).  Each core computes
feat_ext = x_own @ [W | al_blk | ar_blk] for its 12500 nodes, writes a
G-row [256 feat bf16 | 4 el f32(bitcast)] per node plus an er entry for
A-block nodes; G and ER are AllGathered.  Edges (sorted by dst) are
processed in 128-edge tiles: indirect-DMA row gathers of G[src] and
ER[dst], edge softmax numerator ee = exp(leakyrelu(el+er)), and a
0/1 segment-matrix matmul accumulates u = sum(ee*feat), s = sum(ee)
per dst into PSUM windows of 128 dsts.  h = elu(u/s + b1) feeds layer 2
(same structure, mean over heads at the end).
"""
import sys
sys.path.insert(0, '/opt/trn_rl_repo')

import numpy as np
import ml_dtypes

import concourse.bass as bass
import concourse.tile as tile
from concourse import bacc, mybir
from concourse.masks import make_identity

P = 128
NCORES = 8
N0, N1, N2 = 100000, 50000, 8000
E0, E1 = 600000, 80000
F_IN, HID, H, C = 256, 64, 4, 47
NEG = 0.2

BLK1 = N1 // NCORES            # 6250  A/B block size
LPC1 = 2 * BLK1                # 12500 nodes owned per core
LP1 = 12544                    # padded to 98*128
W1N = 49                       # L1 windows per core (6272 dst slots)
DPC1 = W1N * P                 # 6272
BLK2 = N2 // NCORES            # 1000 dst2 per core
W2N = 8                        # L2 windows per core (1024 slots)
DPC2 = W2N * P                 # 1024
GROW1 = 272                    # 256 feat bf16 + 8 el-bitcast + 8 pad
GROW2 = 208                    # 188 feat bf16 + 8 el-bitcast + 12 pad
ER2ROWS = 8064                 # 63*128

F32 = mybir.dt.float32
BF16 = mybir.dt.bfloat16
I32 = mybir.dt.int32
AF = mybir.ActivationFunctionType
OP = mybir.AluOpType
BF = ml_dtypes.bfloat16

_cache = {}


def _g1_row(n):
    """Global node id (layer1 src space, 0..N0) -> G row."""
    m = n % N1
    r = m // BLK1
    return LP1 * r + (m - BLK1 * r) + np.where(n < N1, 0, BLK1)


def _er_row(d):
    """dst1 (0..N1) -> ER row."""
    r = d // BLK1
    return DPC1 * r + (d - BLK1 * r)


def _g2_row(n):
    """node id (layer2 src space, 0..N1) -> G2 row."""
    r = n // BLK1
    return DPC1 * r + (n - BLK1 * r)


def _pack_edges(src_rows, dst, dst_lo, n_dst_local, er_rows, tw):
    """Bucket edges of one core (dst in [dst_lo, dst_lo+n_dst_local)) into
    windows of 128 dsts x tw tiles. Returns meta arrays [P, W*tw]."""
    nw = (n_dst_local + P - 1) // P
    T = nw * tw
    msrc = np.zeros((P, T), np.int32)
    mer = np.zeros((P, T), np.int32)
    mrd = np.zeros((P, T), np.float32)
    mmask = np.zeros((P, T), np.float32)
    dl = dst - dst_lo
    order = np.argsort(dl, kind='stable')
    dl = dl[order]
    sr = src_rows[order]
    err = er_rows[order]
    wofs = dl // P
    bounds = np.searchsorted(wofs, np.arange(nw + 1))
    for w in range(nw):
        a, b = bounds[w], bounds[w + 1]
        n = b - a
        cap = tw * P
        assert n <= cap, (n, cap)
        bs = np.zeros(cap, np.int32); bs[:n] = sr[a:b]
        be = np.zeros(cap, np.int32); be[:n] = err[a:b]
        br = np.zeros(cap, np.float32); br[:n] = (dl[a:b] - w * P)
        bm = np.zeros(cap, np.float32); bm[:n] = 1.0
        cols = slice(w * tw, (w + 1) * tw)
        msrc[:, cols] = bs.reshape(tw, P).T
        mer[:, cols] = be.reshape(tw, P).T
        mrd[:, cols] = br.reshape(tw, P).T
        mmask[:, cols] = bm.reshape(tw, P).T
    return msrc, mer, mrd, mmask


def _edge_phase(nc, tc, pools, G, ER, ms, me, mr, mm, iota_f, nwin, tw, grow,
                nfeat, acc_cols, deps_g, deps_er, flush_fn):
    """Shared L1/L2 edge-processing loop. flush_fn(w, acc) handles PSUM flush."""
    gp, erp, eep, wfp, mp, ps = pools
    for w in range(nwin):
        acc = ps.tile([P, acc_cols], F32, tag="acc")
        gb = gp.tile([P, tw, grow], BF16, tag="g")
        ees = wfp.tile([P, tw, acc_cols], BF16, tag="wf")
        eef = eep.tile([P, tw, 4], F32, tag="ee")
        erb = erp.tile([P, tw, 4], F32, tag="er")
        for j in range(tw):
            t = w * tw + j
            i1 = nc.gpsimd.indirect_dma_start(
                out=gb[:, j, :], out_offset=None, in_=G[:],
                in_offset=bass.IndirectOffsetOnAxis(ap=ms[:, t:t + 1], axis=0))
            i2 = nc.gpsimd.indirect_dma_start(
                out=erb[:, j, :], out_offset=None, in_=ER[:],
                in_offset=bass.IndirectOffsetOnAxis(ap=me[:, t:t + 1], axis=0))
            for d in deps_g:
                tile.add_dep_helper(i1.ins, d.ins, sync=True)
            for d in deps_er:
                tile.add_dep_helper(i2.ins, d.ins, sync=True)
        elv = gb[:].bitcast(F32)  # [P, tw, grow//2]
        eloff = nfeat // 2
        nc.vector.tensor_tensor(out=eef[:], in0=elv[:, :, eloff:eloff + 4],
                                in1=erb[:], op=OP.add)
        ee2 = eep.tile([P, tw, 4], F32, tag="ee2")
        nc.vector.tensor_scalar_mul(out=ee2[:], in0=eef[:], scalar1=NEG)
        nc.vector.tensor_tensor(out=ee2[:], in0=ee2[:], in1=eef[:], op=OP.max)
        nc.scalar.activation(out=ee2[:], in_=ee2[:], func=AF.Exp)
        maskv = mm[:, w * tw:(w + 1) * tw]
        nc.vector.tensor_tensor(out=ee2[:], in0=ee2[:],
                                in1=maskv[:, :, None].broadcast_to([P, tw, 4]),
                                op=OP.mult)
        nc.vector.tensor_copy(out=ees[:, :, nfeat:nfeat + 4], in_=ee2[:])
        for j in range(tw):
            nc.vector.tensor_tensor(
                out=ees[:, j, 0:nfeat].rearrange("p (h d) -> p h d", h=H),
                in0=gb[:, j, 0:nfeat].rearrange("p (h d) -> p h d", h=H),
                in1=ees[:, j, nfeat:nfeat + 4][:, :, None].broadcast_to(
                    [P, H, nfeat // H]),
                op=OP.mult)
            M = mp.tile([P, P], BF16, tag=f"m{j % 2}")
            nc.vector.tensor_tensor(
                out=M[:], in0=iota_f[:],
                in1=mr[:, w * tw + j:w * tw + j + 1].to_broadcast([P, P]),
                op=OP.is_equal)
            nc.tensor.matmul(out=acc[:], lhsT=M[:], rhs=ees[:, j, :],
                             start=(j == 0), stop=(j == tw - 1))
        flush_fn(w, acc)


def build_program(tw1, tw2, add_b1, add_b2):
    key = (tw1, tw2, add_b1, add_b2)
    if key in _cache:
        return _cache[key]
    T1 = W1N * tw1
    T2 = W2N * tw2
    nc = bacc.Bacc("TRN2", num_devices=NCORES)
    # ---- I/O
    xT = nc.declare_dram_parameter("xT", [F_IN, LP1], BF16, isOutput=False)
    W1e = nc.declare_dram_parameter("W1e", [F_IN, 264], BF16, isOutput=False)
    W2e = nc.declare_dram_parameter("W2e", [F_IN, 196], BF16, isOutput=False)
    ar2r = nc.declare_dram_parameter("ar2r", [P, H * C], BF16, isOutput=False)
    b1r = nc.declare_dram_parameter("b1r", [P, 256], F32, isOutput=False)
    b2r = nc.declare_dram_parameter("b2r", [P, C], F32, isOutput=False)
    m1s = nc.declare_dram_parameter("m1s", [P, T1], I32, isOutput=False)
    m1e = nc.declare_dram_parameter("m1e", [P, T1], I32, isOutput=False)
    m1r = nc.declare_dram_parameter("m1r", [P, T1], F32, isOutput=False)
    m1m = nc.declare_dram_parameter("m1m", [P, T1], F32, isOutput=False)
    m2s = nc.declare_dram_parameter("m2s", [P, T2], I32, isOutput=False)
    m2e = nc.declare_dram_parameter("m2e", [P, T2], I32, isOutput=False)
    m2r = nc.declare_dram_parameter("m2r", [P, T2], F32, isOutput=False)
    m2m = nc.declare_dram_parameter("m2m", [P, T2], F32, isOutput=False)
    e2i = nc.declare_dram_parameter("e2i", [P, ER2ROWS // P], I32, isOutput=False)
    OUT = nc.declare_dram_parameter("OUT", [DPC2, C], F32, isOutput=True)
    # ---- internal DRAM
    Gin = nc.dram_tensor("Gin", [LP1, GROW1], BF16)
    ERin = nc.dram_tensor("ERin", [DPC1, 4], F32)
    G = nc.dram_tensor("G", [NCORES * LP1, GROW1], BF16, addr_space="Shared")
    ER = nc.dram_tensor("ER", [NCORES * DPC1, 4], F32, addr_space="Shared")
    G2in = nc.dram_tensor("G2in", [DPC1, GROW2], BF16)
    G2 = nc.dram_tensor("G2", [NCORES * DPC1, GROW2], BF16, addr_space="Shared")
    ER2 = nc.dram_tensor("ER2", [ER2ROWS, 4], F32)

    with tile.TileContext(nc) as tc:
        with (
            tc.tile_pool(name="const", bufs=1) as const,
            tc.tile_pool(name="ps", bufs=2, space="PSUM") as ps,
            tc.tile_pool(name="sb", bufs=3) as sb,
        ):
            iota_i = const.tile([P, P], I32)
            nc.gpsimd.iota(iota_i[:], pattern=[[1, P]], base=0, channel_multiplier=0)
            iota_f = const.tile([P, P], F32)
            nc.vector.tensor_copy(out=iota_f[:], in_=iota_i[:])
            ident = const.tile([P, P], BF16)
            make_identity(nc, ident[:])
            w1t = [const.tile([P, 264], BF16, name=f'w1t{k}', tag=f'w1t{k}') for k in range(2)]
            w2t = [const.tile([P, 196], BF16, name=f'w2t{k}', tag=f'w2t{k}') for k in range(2)]
            for k in range(2):
                nc.sync.dma_start(out=w1t[k][:], in_=W1e[k * P:(k + 1) * P, :])
                nc.sync.dma_start(out=w2t[k][:], in_=W2e[k * P:(k + 1) * P, :])
            b1t = const.tile([P, 256], F32)
            nc.sync.dma_start(out=b1t[:], in_=b1r[:])
            b2t = const.tile([P, C], F32)
            nc.sync.dma_start(out=b2t[:], in_=b2r[:])
            ar2t = const.tile([P, H * C], BF16)
            nc.sync.dma_start(out=ar2t[:], in_=ar2r[:])
            # metadata resident
            ms1 = const.tile([P, T1], I32); nc.sync.dma_start(out=ms1[:], in_=m1s[:])
            me1 = const.tile([P, T1], I32); nc.sync.dma_start(out=me1[:], in_=m1e[:])
            mr1 = const.tile([P, T1], F32); nc.sync.dma_start(out=mr1[:], in_=m1r[:])
            mm1 = const.tile([P, T1], F32); nc.sync.dma_start(out=mm1[:], in_=m1m[:])
            ms2 = const.tile([P, T2], I32); nc.sync.dma_start(out=ms2[:], in_=m2s[:])
            me2 = const.tile([P, T2], I32); nc.sync.dma_start(out=me2[:], in_=m2e[:])
            mr2 = const.tile([P, T2], F32); nc.sync.dma_start(out=mr2[:], in_=m2r[:])
            mm2 = const.tile([P, T2], F32); nc.sync.dma_start(out=mm2[:], in_=m2m[:])
            e2it = const.tile([P, ER2ROWS // P], I32)
            nc.sync.dma_start(out=e2it[:], in_=e2i[:])

            # ================= phase 1: feat1 = x @ W1e =================
            g_writes = []
            with tc.tile_pool(name="xp", bufs=1) as xp:
                xt = [xp.tile([P, LP1], BF16, name=f'xt{k}', tag=f'xt{k}') for k in range(2)]
                for k in range(2):
                    nc.sync.dma_start(out=xt[k][:], in_=xT[k * P:(k + 1) * P, :])
                for c in range(LP1 // P):
                    pm = ps.tile([P, 264], F32, tag="pfeat")
                    for k in range(2):
                        nc.tensor.matmul(out=pm[:],
                                         lhsT=xt[k][:, c * P:(c + 1) * P],
                                         rhs=w1t[k][:],
                                         start=(k == 0), stop=(k == 1))
                    gs = sb.tile([P, GROW1], BF16, tag="gs")
                    nc.vector.tensor_copy(out=gs[:, 0:256], in_=pm[:, 0:256])
                    nc.vector.tensor_copy(
                        out=gs[:, 256:264].bitcast(F32)[:, 0:4], in_=pm[:, 256:260])
                    d1 = nc.sync.dma_start(out=Gin[c * P:(c + 1) * P, :], in_=gs[:])
                    g_writes.append(d1)
                    if c < W1N:  # er rows 0..6272
                        es = sb.tile([P, 4], F32, tag="es")
                        nc.vector.tensor_copy(out=es[:], in_=pm[:, 260:264])
                        d2 = nc.sync.dma_start(out=ERin[c * P:(c + 1) * P, :], in_=es[:])
                        g_writes.append(d2)

            # ================= phase 2: AllGather =================
            cc1 = nc.gpsimd.collective_compute(
                "AllGather", OP.bypass, replica_groups=[list(range(NCORES))],
                ins=[Gin[:]], outs=[G[:]])
            cc2 = nc.gpsimd.collective_compute(
                "AllGather", OP.bypass, replica_groups=[list(range(NCORES))],
                ins=[ERin[:]], outs=[ER[:]])
            for d in g_writes:
                tile.add_dep_helper(cc1.ins, d.ins, sync=True)
                tile.add_dep_helper(cc2.ins, d.ins, sync=True)

            # ================= phase 3: L1 edge phase =================
            hT = [const.tile([P, DPC1], BF16, name=f'hT{k}', tag=f'hT{k}') for k in range(2)]

            def flush1(w, acc):
                sden = sb.tile([P, 4], F32, tag="sden")
                nc.vector.tensor_scalar_max(out=sden[:], in0=acc[:, 256:260],
                                            scalar1=1e-30)
                nc.vector.reciprocal(out=sden[:], in_=sden[:])
                z = sb.tile([P, 256], F32, tag="z")
                nc.vector.tensor_tensor(
                    out=z[:].rearrange("p (h d) -> p h d", h=H),
                    in0=acc[:, 0:256].rearrange("p (h d) -> p h d", h=H),
                    in1=sden[:, :, None].broadcast_to([P, H, HID]), op=OP.mult)
                if add_b1:
                    nc.vector.tensor_tensor(out=z[:], in0=z[:], in1=b1t[:], op=OP.add)
                # elu: h = (max(z,0)-1) + exp(min(z,0))
                zm = sb.tile([P, 256], F32, tag="zm")
                nc.vector.tensor_scalar_min(out=zm[:], in0=z[:], scalar1=0.0)
                nc.scalar.activation(out=zm[:], in_=zm[:], func=AF.Exp)
                hb = sb.tile([P, 256], BF16, tag="hb")
                nc.vector.tensor_scalar(out=hb[:], in0=z[:], scalar1=0.0,
                                        scalar2=-1.0, op0=OP.max, op1=OP.add)
                nc.vector.tensor_tensor(out=hb[:], in0=hb[:], in1=zm[:], op=OP.add)
                # transpose 2x [128,128] -> hT k-tiles
                for k in range(2):
                    tp = ps.tile([P, P], BF16, tag="tp")
                    nc.tensor.transpose(out=tp[:], in_=hb[:, k * P:(k + 1) * P],
                                        identity=ident[:])
                    nc.vector.tensor_copy(out=hT[k][:, w * P:(w + 1) * P], in_=tp[:])

            with (
                tc.tile_pool(name="gp", bufs=2) as gp,
                tc.tile_pool(name="erp", bufs=2) as erp,
                tc.tile_pool(name="eep", bufs=2) as eep,
                tc.tile_pool(name="wfp", bufs=2) as wfp,
                tc.tile_pool(name="mp", bufs=4) as mp,
            ):
                _edge_phase(nc, tc, (gp, erp, eep, wfp, mp, ps),
                            G, ER, ms1, me1, mr1, mm1, iota_f,
                            W1N, tw1, GROW1, 256, 260, [cc1], [cc2], flush1)

            # ================= phase 4: feat2 = h @ W2e =================
            g2_writes = []
            for c in range(W1N):
                pm = ps.tile([P, 196], F32, tag="pfeat2")
                for k in range(2):
                    nc.tensor.matmul(out=pm[:],
                                     lhsT=hT[k][:, c * P:(c + 1) * P],
                                     rhs=w2t[k][:],
                                     start=(k == 0), stop=(k == 1))
                gs = sb.tile([P, GROW2], BF16, tag="gs2")
                nc.vector.tensor_copy(out=gs[:, 0:188], in_=pm[:, 0:188])
                nc.vector.tensor_copy(
                    out=gs[:, 188:196].bitcast(F32)[:, 0:4], in_=pm[:, 188:192])
                nc.gpsimd.memset(gs[:, 196:208], 0)
                d1 = nc.sync.dma_start(out=G2in[c * P:(c + 1) * P, :], in_=gs[:])
                g2_writes.append(d1)

            # ================= phase 5: AllGather G2 =================
            cc3 = nc.gpsimd.collective_compute(
                "AllGather", OP.bypass, replica_groups=[list(range(NCORES))],
                ins=[G2in[:]], outs=[G2[:]])
            for d in g2_writes:
                tile.add_dep_helper(cc3.ins, d.ins, sync=True)

            # ======== phase 5b: er2 = feat2[dst2] . ar2  (redundant) ========
            er2_writes = []
            with tc.tile_pool(name="e2p", bufs=3) as e2p:
                for c in range(ER2ROWS // P):
                    g2c = e2p.tile([P, GROW2], BF16, tag="g2c")
                    i1 = nc.gpsimd.indirect_dma_start(
                        out=g2c[:], out_offset=None, in_=G2[:],
                        in_offset=bass.IndirectOffsetOnAxis(ap=e2it[:, c:c + 1], axis=0))
                    tile.add_dep_helper(i1.ins, cc3.ins, sync=True)
                    t1 = e2p.tile([P, H * C], F32, tag="t1")
                    nc.vector.tensor_tensor(out=t1[:], in0=g2c[:, 0:H * C],
                                            in1=ar2t[:], op=OP.mult)
                    t2 = e2p.tile([P, 4], F32, tag="t2")
                    nc.vector.tensor_reduce(out=t2[:], in_=t1[:].rearrange(
                        "p (h c) -> p h c", h=H), axis=mybir.AxisListType.X, op=OP.add)
                    d1 = nc.sync.dma_start(out=ER2[c * P:(c + 1) * P, :], in_=t2[:])
                    er2_writes.append(d1)

            # ================= phase 6: L2 edge phase =================
            def flush2(w, acc):
                sden = sb.tile([P, 4], F32, tag="sden2")
                nc.vector.tensor_scalar_max(out=sden[:], in0=acc[:, 188:192],
                                            scalar1=1e-30)
                nc.vector.reciprocal(out=sden[:], in_=sden[:])
                nc.vector.tensor_scalar_mul(out=sden[:], in0=sden[:], scalar1=0.25)
                z = sb.tile([P, 188], F32, tag="z2")
                nc.vector.tensor_tensor(
                    out=z[:].rearrange("p (h c) -> p h c", h=H),
                    in0=acc[:, 0:188].rearrange("p (h c) -> p h c", h=H),
                    in1=sden[:, :, None].broadcast_to([P, H, C]), op=OP.mult)
                o = sb.tile([P, C], F32, tag="o")
                nc.vector.tensor_reduce(
                    out=o[:], in_=z[:].rearrange("p (h c) -> p c h", h=H),
                    axis=mybir.AxisListType.X, op=OP.add)
                if add_b2:
                    nc.vector.tensor_tensor(out=o[:], in0=o[:], in1=b2t[:], op=OP.add)
                nc.sync.dma_start(out=OUT[w * P:(w + 1) * P, :], in_=o[:])

            with (
                tc.tile_pool(name="gp2", bufs=2) as gp2,
                tc.tile_pool(name="erp2", bufs=2) as erp2,
                tc.tile_pool(name="eep2", bufs=2) as eep2,
                tc.tile_pool(name="wfp2", bufs=2) as wfp2,
                tc.tile_pool(name="mp2", bufs=4) as mp2,
            ):
                _edge_phase(nc, tc, (gp2, erp2, eep2, wfp2, mp2, ps),
                            G2, ER2, ms2, me2, mr2, mm2, iota_f,
                            W2N, tw2, GROW2, 188, 192, [cc3], er2_writes, flush2)

    nc.compile()
    _cache[key] = nc
    return nc


def _f32_to_bf16_slots(a):
    """[N,4] f32 -> [N,8] raw bf16 slots (bitcast)."""
    return a.view(np.uint16).reshape(a.shape[0], 8).view(BF)


def kernel(x, W1, al1, ar1, b1, W2, al2, ar2, b2, src0, dst0, src1, dst1):
    x = np.asarray(x, np.float32); W1 = np.asarray(W1, np.float32)
    al1 = np.asarray(al1, np.float32); ar1 = np.asarray(ar1, np.float32)
    b1 = np.asarray(b1, np.float32); W2 = np.asarray(W2, np.float32)
    al2 = np.asarray(al2, np.float32); ar2 = np.asarray(ar2, np.float32)
    b2 = np.asarray(b2, np.float32)
    src0 = np.asarray(src0, np.int32); dst0 = np.asarray(dst0, np.int32)
    src1 = np.asarray(src1, np.int32); dst1 = np.asarray(dst1, np.int32)

    # ---- weight extensions (block-diagonal attention columns)
    def blkdiag(a):  # [H, D] -> [H*D, H]
        out = np.zeros((a.shape[0] * a.shape[1], a.shape[0]), np.float32)
        for h in range(a.shape[0]):
            out[h * a.shape[1]:(h + 1) * a.shape[1], h] = a[h]
        return out

    W1e = np.concatenate([W1, W1 @ blkdiag(al1), W1 @ blkdiag(ar1)],
                         axis=1).astype(BF)
    W2e = np.concatenate([W2, W2 @ blkdiag(al2), W2 @ blkdiag(ar2)],
                         axis=1).astype(BF)
    ar2r = np.broadcast_to(ar2.reshape(1, H * C), (P, H * C)).astype(BF).copy()
    b1r = np.broadcast_to(b1.reshape(1, 256), (P, 256)).astype(np.float32).copy()
    b2m = b2.reshape(H, C).mean(axis=0)
    b2r = np.broadcast_to(b2m.reshape(1, C), (P, C)).astype(np.float32).copy()
    add_b1 = bool(np.any(b1)); add_b2 = bool(np.any(b2))

    # ---- per-core edge metadata
    g1 = _g1_row(src0)
    er1 = _er_row(dst0)
    g2 = _g2_row(src1)
    core1 = dst0 // BLK1
    core2 = dst1 // BLK2
    # choose tile counts (max over cores/windows)
    def max_tw(dst, core, blk, nwin):
        mx = 1
        for r in range(NCORES):
            dl = dst[core == r] - r * blk
            cnt = np.bincount(dl // P, minlength=nwin)
            mx = max(mx, int(np.ceil(cnt.max() / P)))
        return mx
    tw1 = max_tw(dst0, core1, BLK1, W1N)
    tw2 = max_tw(dst1, core2, BLK2, W2N)

    er2_idx = _g2_row(np.minimum(np.arange(ER2ROWS), N1 - 1)).astype(np.int32)
    er2_idx = er2_idx.reshape(ER2ROWS // P, P).T.copy()  # [P, chunks]

    in_maps = []
    for r in range(NCORES):
        sel1 = core1 == r
        m1s_, m1e_, m1r_, m1m_ = _pack_edges(
            g1[sel1], dst0[sel1], r * BLK1, BLK1, er1[sel1], tw1)
        sel2 = core2 == r
        m2s_, m2e_, m2r_, m2m_ = _pack_edges(
            g2[sel2], dst1[sel2], r * BLK2, BLK2,
            np.minimum(dst1[sel2], ER2ROWS - 1), tw2)
        rows = np.concatenate([
            np.arange(r * BLK1, (r + 1) * BLK1),
            np.arange(N1 + r * BLK1, N1 + (r + 1) * BLK1)])
        xT_ = np.zeros((F_IN, LP1), BF)
        xT_[:, :LPC1] = x[rows].T.astype(BF)
        in_maps.append(dict(
            xT=xT_, W1e=W1e, W2e=W2e, ar2r=ar2r, b1r=b1r, b2r=b2r,
            m1s=m1s_, m1e=m1e_, m1r=m1r_, m1m=m1m_,
            m2s=m2s_, m2e=m2e_, m2r=m2r_, m2m=m2m_, e2i=er2_idx))

    global _last_in_maps
    _last_in_maps = in_maps
    nc = build_program(tw1, tw2, add_b1, add_b2)
    from concourse.bass_utils import run_bass_kernel_spmd
    res = run_bass_kernel_spmd(nc, in_maps, core_ids=list(range(NCORES)))
    out = np.concatenate([res.results[r]["OUT"][:BLK2] for r in range(NCORES)],
                         axis=0)
    return out.astype(np.float32)


# revision 7
# speedup vs baseline: 75.2503x; 75.2503x over previous
"""DGL-style 2-layer GAT on 8 TRN2 NeuronCores (Bass/Tile).

Sharding (per sharding_hint): dst nodes + incident edges partitioned
across 8 cores; weights replicated; src features shared via AllGather.

Node ownership: core r owns node rows A_r=[6250r,6250(r+1)) and
B_r=[50000+6250r, 50000+6250(r+1)).  Each core computes
feat_ext = x_own @ [W | al_blk | ar_blk] for its 12500 nodes, writes a
G-row [256 feat bf16 | 4 el f32(bitcast)] per node plus an er entry for
A-block nodes; G and ER are AllGathered.  Edges (sorted by dst) are
processed in 128-edge tiles: indirect-DMA row gathers of G[src] and
ER[dst], edge softmax numerator ee = exp(leakyrelu(el+er)), and a
0/1 segment-matrix matmul accumulates u = sum(ee*feat), s = sum(ee)
per dst into PSUM windows of 128 dsts.  h = elu(u/s + b1) feeds layer 2
(same structure, mean over heads at the end).
"""
import sys
sys.path.insert(0, '/opt/trn_rl_repo')

import numpy as np
import ml_dtypes

import concourse.bass as bass
import concourse.tile as tile
from concourse import bacc, mybir
from concourse.masks import make_identity

P = 128
NCORES = 8
N0, N1, N2 = 100000, 50000, 8000
E0, E1 = 600000, 80000
F_IN, HID, H, C = 256, 64, 4, 47
NEG = 0.2

BLK1 = N1 // NCORES            # 6250  A/B block size
LPC1 = 2 * BLK1                # 12500 nodes owned per core
LP1 = 12544                    # padded to 98*128
W1N = 49                       # L1 windows per core (6272 dst slots)
DPC1 = W1N * P                 # 6272
BLK2 = N2 // NCORES            # 1000 dst2 per core
W2N = 8                        # L2 windows per core (1024 slots)
DPC2 = W2N * P                 # 1024
GROW1 = 272                    # 256 feat bf16 + 8 el-bitcast + 8 pad
GROW2 = 208                    # 188 feat bf16 + 8 el-bitcast + 12 pad
ER2ROWS = 8064                 # 63*128

F32 = mybir.dt.float32
BF16 = mybir.dt.bfloat16
I32 = mybir.dt.int32
AF = mybir.ActivationFunctionType
OP = mybir.AluOpType
BF = ml_dtypes.bfloat16

_cache = {}


def _g1_row(n):
    """Global node id (layer1 src space, 0..N0) -> G row."""
    m = n % N1
    r = m // BLK1
    return LP1 * r + (m - BLK1 * r) + np.where(n < N1, 0, BLK1)


def _er_row(d):
    """dst1 (0..N1) -> ER row."""
    r = d // BLK1
    return DPC1 * r + (d - BLK1 * r)


def _g2_row(n):
    """node id (layer2 src space, 0..N1) -> G2 row."""
    r = n // BLK1
    return DPC1 * r + (n - BLK1 * r)


def _pack_edges(src_rows, dst, dst_lo, n_dst_local, er_rows, tw):
    """Bucket edges of one core (dst in [dst_lo, dst_lo+n_dst_local)) into
    windows of 128 dsts x tw tiles. Returns meta arrays [P, W*tw]."""
    nw = (n_dst_local + P - 1) // P
    T = nw * tw
    msrc = np.zeros((P, T), np.int32)
    mer = np.zeros((P, T), np.int32)
    mrd = np.zeros((P, T), np.float32)
    mmask = np.zeros((P, T), np.float32)
    dl = dst - dst_lo
    order = np.argsort(dl, kind='stable')
    dl = dl[order]
    sr = src_rows[order]
    err = er_rows[order]
    wofs = dl // P
    bounds = np.searchsorted(wofs, np.arange(nw + 1))
    for w in range(nw):
        a, b = bounds[w], bounds[w + 1]
        n = b - a
        cap = tw * P
        assert n <= cap, (n, cap)
        bs = np.zeros(cap, np.int32); bs[:n] = sr[a:b]
        be = np.zeros(cap, np.int32); be[:n] = err[a:b]
        br = np.zeros(cap, np.float32); br[:n] = (dl[a:b] - w * P)
        bm = np.zeros(cap, np.float32); bm[:n] = 1.0
        cols = slice(w * tw, (w + 1) * tw)
        msrc[:, cols] = bs.reshape(tw, P).T
        mer[:, cols] = be.reshape(tw, P).T
        mrd[:, cols] = br.reshape(tw, P).T
        mmask[:, cols] = bm.reshape(tw, P).T
    return msrc, mer, mrd, mmask


def _edge_phase(nc, tc, pools, G, ER, ms, me, mr, mm, iota_f, nwin, tw, grow,
                nfeat, acc_cols, deps_g, deps_er, flush_fn):
    """Shared L1/L2 edge-processing loop. flush_fn(w, acc) handles PSUM flush."""
    gp, erp, eep, wfp, mp, ps = pools
    for w in range(nwin):
        acc = ps.tile([P, acc_cols], F32, tag="acc")
        gb = gp.tile([P, tw, grow], BF16, tag="g")
        ees = wfp.tile([P, tw, acc_cols], BF16, tag="wf")
        eef = eep.tile([P, tw, 4], F32, tag="ee")
        erb = erp.tile([P, tw, 4], F32, tag="er")
        for j in range(tw):
            t = w * tw + j
            i1 = nc.gpsimd.indirect_dma_start(
                out=gb[:, j, :], out_offset=None, in_=G[:],
                in_offset=bass.IndirectOffsetOnAxis(ap=ms[:, t:t + 1], axis=0))
            i2 = nc.gpsimd.indirect_dma_start(
                out=erb[:, j, :], out_offset=None, in_=ER[:],
                in_offset=bass.IndirectOffsetOnAxis(ap=me[:, t:t + 1], axis=0))
            for d in deps_g:
                tile.add_dep_helper(i1.ins, d.ins, sync=True)
            for d in deps_er:
                tile.add_dep_helper(i2.ins, d.ins, sync=True)
        elv = gb[:].bitcast(F32)  # [P, tw, grow//2]
        eloff = nfeat // 2
        nc.vector.tensor_tensor(out=eef[:], in0=elv[:, :, eloff:eloff + 4],
                                in1=erb[:], op=OP.add)
        ee2 = eep.tile([P, tw, 4], F32, tag="ee2")
        nc.vector.tensor_scalar_mul(out=ee2[:], in0=eef[:], scalar1=NEG)
        nc.vector.tensor_tensor(out=ee2[:], in0=ee2[:], in1=eef[:], op=OP.max)
        nc.scalar.activation(out=ee2[:], in_=ee2[:], func=AF.Exp)
        maskv = mm[:, w * tw:(w + 1) * tw]
        nc.vector.tensor_tensor(out=ee2[:], in0=ee2[:],
                                in1=maskv[:, :, None].broadcast_to([P, tw, 4]),
                                op=OP.mult)
        nc.vector.tensor_copy(out=ees[:, :, nfeat:nfeat + 4], in_=ee2[:])
        for j in range(tw):
            nc.vector.tensor_tensor(
                out=ees[:, j, 0:nfeat].rearrange("p (h d) -> p h d", h=H),
                in0=gb[:, j, 0:nfeat].rearrange("p (h d) -> p h d", h=H),
                in1=ees[:, j, nfeat:nfeat + 4][:, :, None].broadcast_to(
                    [P, H, nfeat // H]),
                op=OP.mult)
            M = mp.tile([P, P], BF16, tag=f"m{j % 2}")
            nc.vector.tensor_tensor(
                out=M[:], in0=iota_f[:],
                in1=mr[:, w * tw + j:w * tw + j + 1].to_broadcast([P, P]),
                op=OP.is_equal)
            nc.tensor.matmul(out=acc[:], lhsT=M[:], rhs=ees[:, j, :],
                             start=(j == 0), stop=(j == tw - 1))
        flush_fn(w, acc)


def build_program(tw1, tw2, add_b1, add_b2):
    key = (tw1, tw2, add_b1, add_b2)
    if key in _cache:
        return _cache[key]
    T1 = W1N * tw1
    T2 = W2N * tw2
    nc = bacc.Bacc("TRN2", num_devices=NCORES)
    # ---- I/O
    xT = nc.declare_dram_parameter("xT", [F_IN, LP1], BF16, isOutput=False)
    W1e = nc.declare_dram_parameter("W1e", [F_IN, 264], BF16, isOutput=False)
    W2e = nc.declare_dram_parameter("W2e", [F_IN, 196], BF16, isOutput=False)
    ar2r = nc.declare_dram_parameter("ar2r", [P, H * C], BF16, isOutput=False)
    b1r = nc.declare_dram_parameter("b1r", [P, 256], F32, isOutput=False)
    b2r = nc.declare_dram_parameter("b2r", [P, C], F32, isOutput=False)
    m1s = nc.declare_dram_parameter("m1s", [P, T1], I32, isOutput=False)
    m1e = nc.declare_dram_parameter("m1e", [P, T1], I32, isOutput=False)
    m1r = nc.declare_dram_parameter("m1r", [P, T1], F32, isOutput=False)
    m1m = nc.declare_dram_parameter("m1m", [P, T1], F32, isOutput=False)
    m2s = nc.declare_dram_parameter("m2s", [P, T2], I32, isOutput=False)
    m2e = nc.declare_dram_parameter("m2e", [P, T2], I32, isOutput=False)
    m2r = nc.declare_dram_parameter("m2r", [P, T2], F32, isOutput=False)
    m2m = nc.declare_dram_parameter("m2m", [P, T2], F32, isOutput=False)
    e2i = nc.declare_dram_parameter("e2i", [P, ER2ROWS // P], I32, isOutput=False)
    OUT = nc.declare_dram_parameter("OUT", [DPC2, C], F32, isOutput=True)
    # ---- internal DRAM
    Gin = nc.dram_tensor("Gin", [LP1, GROW1], BF16)
    ERin = nc.dram_tensor("ERin", [DPC1, 4], F32)
    G = nc.dram_tensor("G", [NCORES * LP1, GROW1], BF16, addr_space="Shared")
    ER = nc.dram_tensor("ER", [NCORES * DPC1, 4], F32, addr_space="Shared")
    G2in = nc.dram_tensor("G2in", [DPC1, GROW2], BF16)
    G2 = nc.dram_tensor("G2", [NCORES * DPC1, GROW2], BF16, addr_space="Shared")
    ER2 = nc.dram_tensor("ER2", [ER2ROWS, 4], F32)

    with tile.TileContext(nc) as tc:
        with (
            tc.tile_pool(name="const", bufs=1) as const,
            tc.tile_pool(name="ps", bufs=2, space="PSUM") as ps,
            tc.tile_pool(name="sb", bufs=3) as sb,
        ):
            iota_i = const.tile([P, P], I32)
            nc.gpsimd.iota(iota_i[:], pattern=[[1, P]], base=0, channel_multiplier=0)
            iota_f = const.tile([P, P], F32)
            nc.vector.tensor_copy(out=iota_f[:], in_=iota_i[:])
            ident = const.tile([P, P], BF16)
            make_identity(nc, ident[:])
            w1t = [const.tile([P, 264], BF16, name=f'w1t{k}', tag=f'w1t{k}') for k in range(2)]
            w2t = [const.tile([P, 196], BF16, name=f'w2t{k}', tag=f'w2t{k}') for k in range(2)]
            for k in range(2):
                nc.sync.dma_start(out=w1t[k][:], in_=W1e[k * P:(k + 1) * P, :])
                nc.sync.dma_start(out=w2t[k][:], in_=W2e[k * P:(k + 1) * P, :])
            b1t = const.tile([P, 256], F32)
            nc.sync.dma_start(out=b1t[:], in_=b1r[:])
            b2t = const.tile([P, C], F32)
            nc.sync.dma_start(out=b2t[:], in_=b2r[:])
            ar2t = const.tile([P, H * C], BF16)
            nc.sync.dma_start(out=ar2t[:], in_=ar2r[:])
            # metadata resident
            ms1 = const.tile([P, T1], I32); nc.sync.dma_start(out=ms1[:], in_=m1s[:])
            me1 = const.tile([P, T1], I32); nc.sync.dma_start(out=me1[:], in_=m1e[:])
            mr1 = const.tile([P, T1], F32); nc.sync.dma_start(out=mr1[:], in_=m1r[:])
            mm1 = const.tile([P, T1], F32); nc.sync.dma_start(out=mm1[:], in_=m1m[:])
            ms2 = const.tile([P, T2], I32); nc.sync.dma_start(out=ms2[:], in_=m2s[:])
            me2 = const.tile([P, T2], I32); nc.sync.dma_start(out=me2[:], in_=m2e[:])
            mr2 = const.tile([P, T2], F32); nc.sync.dma_start(out=mr2[:], in_=m2r[:])
            mm2 = const.tile([P, T2], F32); nc.sync.dma_start(out=mm2[:], in_=m2m[:])
            e2it = const.tile([P, ER2ROWS // P], I32)
            nc.sync.dma_start(out=e2it[:], in_=e2i[:])

            # ================= phase 1: feat1 = x @ W1e =================
            g_writes = []
            with tc.tile_pool(name="xp", bufs=1) as xp:
                xt = [xp.tile([P, LP1], BF16, name=f'xt{k}', tag=f'xt{k}') for k in range(2)]
                for k in range(2):
                    nc.sync.dma_start(out=xt[k][:], in_=xT[k * P:(k + 1) * P, :])
                for c in range(LP1 // P):
                    pm = ps.tile([P, 264], F32, tag="pfeat")
                    for k in range(2):
                        nc.tensor.matmul(out=pm[:],
                                         lhsT=xt[k][:, c * P:(c + 1) * P],
                                         rhs=w1t[k][:],
                                         start=(k == 0), stop=(k == 1))
                    gs = sb.tile([P, GROW1], BF16, tag="gs")
                    nc.vector.tensor_copy(out=gs[:, 0:256], in_=pm[:, 0:256])
                    nc.vector.tensor_copy(
                        out=gs[:, 256:264].bitcast(F32)[:, 0:4], in_=pm[:, 256:260])
                    d1 = nc.sync.dma_start(out=Gin[c * P:(c + 1) * P, :], in_=gs[:])
                    g_writes.append(d1)
                    if c < W1N:  # er rows 0..6272
                        es = sb.tile([P, 4], F32, tag="es")
                        nc.vector.tensor_copy(out=es[:], in_=pm[:, 260:264])
                        d2 = nc.sync.dma_start(out=ERin[c * P:(c + 1) * P, :], in_=es[:])
                        g_writes.append(d2)

            # ================= phase 2: AllGather =================
            cc1 = nc.gpsimd.collective_compute(
                "AllGather", OP.bypass, replica_groups=[list(range(NCORES))],
                ins=[Gin[:]], outs=[G[:]])
            cc2 = nc.gpsimd.collective_compute(
                "AllGather", OP.bypass, replica_groups=[list(range(NCORES))],
                ins=[ERin[:]], outs=[ER[:]])
            for d in g_writes:
                tile.add_dep_helper(cc1.ins, d.ins, sync=True)
                tile.add_dep_helper(cc2.ins, d.ins, sync=True)

            # ================= phase 3: L1 edge phase =================
            hT = [const.tile([P, DPC1], BF16, name=f'hT{k}', tag=f'hT{k}') for k in range(2)]

            def flush1(w, acc):
                sden = sb.tile([P, 4], F32, tag="sden")
                nc.vector.tensor_scalar_max(out=sden[:], in0=acc[:, 256:260],
                                            scalar1=1e-30)
                nc.vector.reciprocal(out=sden[:], in_=sden[:])
                z = sb.tile([P, 256], F32, tag="z")
                nc.vector.tensor_tensor(
                    out=z[:].rearrange("p (h d) -> p h d", h=H),
                    in0=acc[:, 0:256].rearrange("p (h d) -> p h d", h=H),
                    in1=sden[:, :, None].broadcast_to([P, H, HID]), op=OP.mult)
                if add_b1:
                    nc.vector.tensor_tensor(out=z[:], in0=z[:], in1=b1t[:], op=OP.add)
                # elu: h = (max(z,0)-1) + exp(min(z,0))
                zm = sb.tile([P, 256], F32, tag="zm")
                nc.vector.tensor_scalar_min(out=zm[:], in0=z[:], scalar1=0.0)
                nc.scalar.activation(out=zm[:], in_=zm[:], func=AF.Exp)
                hb = sb.tile([P, 256], BF16, tag="hb")
                nc.vector.tensor_scalar(out=hb[:], in0=z[:], scalar1=0.0,
                                        scalar2=-1.0, op0=OP.max, op1=OP.add)
                nc.vector.tensor_tensor(out=hb[:], in0=hb[:], in1=zm[:], op=OP.add)
                # transpose 2x [128,128] -> hT k-tiles
                for k in range(2):
                    tp = ps.tile([P, P], BF16, tag="tp")
                    nc.tensor.transpose(out=tp[:], in_=hb[:, k * P:(k + 1) * P],
                                        identity=ident[:])
                    nc.vector.tensor_copy(out=hT[k][:, w * P:(w + 1) * P], in_=tp[:])

            with (
                tc.tile_pool(name="gp", bufs=2) as gp,
                tc.tile_pool(name="erp", bufs=2) as erp,
                tc.tile_pool(name="eep", bufs=2) as eep,
                tc.tile_pool(name="wfp", bufs=2) as wfp,
                tc.tile_pool(name="mp", bufs=4) as mp,
            ):
                _edge_phase(nc, tc, (gp, erp, eep, wfp, mp, ps),
                            G, ER, ms1, me1, mr1, mm1, iota_f,
                            W1N, tw1, GROW1, 256, 260, [cc1], [cc2], flush1)

            # ================= phase 4: feat2 = h @ W2e =================
            g2_writes = []
            for c in range(W1N):
                pm = ps.tile([P, 196], F32, tag="pfeat2")
                for k in range(2):
                    nc.tensor.matmul(out=pm[:],
                                     lhsT=hT[k][:, c * P:(c + 1) * P],
                                     rhs=w2t[k][:],
                                     start=(k == 0), stop=(k == 1))
                gs = sb.tile([P, GROW2], BF16, tag="gs2")
                nc.vector.tensor_copy(out=gs[:, 0:188], in_=pm[:, 0:188])
                nc.vector.tensor_copy(
                    out=gs[:, 188:196].bitcast(F32)[:, 0:4], in_=pm[:, 188:192])
                nc.gpsimd.memset(gs[:, 196:208], 0)
                d1 = nc.sync.dma_start(out=G2in[c * P:(c + 1) * P, :], in_=gs[:])
                g2_writes.append(d1)

            # ================= phase 5: AllGather G2 =================
            cc3 = nc.gpsimd.collective_compute(
                "AllGather", OP.bypass, replica_groups=[list(range(NCORES))],
                ins=[G2in[:]], outs=[G2[:]])
            for d in g2_writes:
                tile.add_dep_helper(cc3.ins, d.ins, sync=True)

            # ======== phase 5b: er2 = feat2[dst2] . ar2  (redundant) ========
            er2_writes = []
            with tc.tile_pool(name="e2p", bufs=3) as e2p:
                for c in range(ER2ROWS // P):
                    g2c = e2p.tile([P, GROW2], BF16, tag="g2c")
                    i1 = nc.gpsimd.indirect_dma_start(
                        out=g2c[:], out_offset=None, in_=G2[:],
                        in_offset=bass.IndirectOffsetOnAxis(ap=e2it[:, c:c + 1], axis=0))
                    tile.add_dep_helper(i1.ins, cc3.ins, sync=True)
                    t1 = e2p.tile([P, H * C], F32, tag="t1")
                    nc.vector.tensor_tensor(out=t1[:], in0=g2c[:, 0:H * C],
                                            in1=ar2t[:], op=OP.mult)
                    t2 = e2p.tile([P, 4], F32, tag="t2")
                    nc.vector.tensor_reduce(out=t2[:], in_=t1[:].rearrange(
                        "p (h c) -> p h c", h=H), axis=mybir.AxisListType.X, op=OP.add)
                    d1 = nc.sync.dma_start(out=ER2[c * P:(c + 1) * P, :], in_=t2[:])
                    er2_writes.append(d1)

            # ================= phase 6: L2 edge phase =================
            def flush2(w, acc):
                sden = sb.tile([P, 4], F32, tag="sden2")
                nc.vector.tensor_scalar_max(out=sden[:], in0=acc[:, 188:192],
                                            scalar1=1e-30)
                nc.vector.reciprocal(out=sden[:], in_=sden[:])
                nc.vector.tensor_scalar_mul(out=sden[:], in0=sden[:], scalar1=0.25)
                z = sb.tile([P, 188], F32, tag="z2")
                nc.vector.tensor_tensor(
                    out=z[:].rearrange("p (h c) -> p h c", h=H),
                    in0=acc[:, 0:188].rearrange("p (h c) -> p h c", h=H),
                    in1=sden[:, :, None].broadcast_to([P, H, C]), op=OP.mult)
                o = sb.tile([P, C], F32, tag="o")
                nc.vector.tensor_reduce(
                    out=o[:], in_=z[:].rearrange("p (h c) -> p c h", h=H),
                    axis=mybir.AxisListType.X, op=OP.add)
                if add_b2:
                    nc.vector.tensor_tensor(out=o[:], in0=o[:], in1=b2t[:], op=OP.add)
                nc.sync.dma_start(out=OUT[w * P:(w + 1) * P, :], in_=o[:])

            with (
                tc.tile_pool(name="gp2", bufs=2) as gp2,
                tc.tile_pool(name="erp2", bufs=2) as erp2,
                tc.tile_pool(name="eep2", bufs=2) as eep2,
                tc.tile_pool(name="wfp2", bufs=2) as wfp2,
                tc.tile_pool(name="mp2", bufs=4) as mp2,
            ):
                _edge_phase(nc, tc, (gp2, erp2, eep2, wfp2, mp2, ps),
                            G2, ER2, ms2, me2, mr2, mm2, iota_f,
                            W2N, tw2, GROW2, 188, 192, [cc3], er2_writes, flush2)

    nc.compile()
    _cache[key] = nc
    return nc


def _f32_to_bf16_slots(a):
    """[N,4] f32 -> [N,8] raw bf16 slots (bitcast)."""
    return a.view(np.uint16).reshape(a.shape[0], 8).view(BF)


def kernel(x, W1, al1, ar1, b1, W2, al2, ar2, b2, src0, dst0, src1, dst1):
    x = np.asarray(x, np.float32); W1 = np.asarray(W1, np.float32)
    al1 = np.asarray(al1, np.float32); ar1 = np.asarray(ar1, np.float32)
    b1 = np.asarray(b1, np.float32); W2 = np.asarray(W2, np.float32)
    al2 = np.asarray(al2, np.float32); ar2 = np.asarray(ar2, np.float32)
    b2 = np.asarray(b2, np.float32)
    src0 = np.asarray(src0, np.int32); dst0 = np.asarray(dst0, np.int32)
    src1 = np.asarray(src1, np.int32); dst1 = np.asarray(dst1, np.int32)

    # ---- weight extensions (block-diagonal attention columns)
    def blkdiag(a):  # [H, D] -> [H*D, H]
        out = np.zeros((a.shape[0] * a.shape[1], a.shape[0]), np.float32)
        for h in range(a.shape[0]):
            out[h * a.shape[1]:(h + 1) * a.shape[1], h] = a[h]
        return out

    W1e = np.concatenate([W1, W1 @ blkdiag(al1), W1 @ blkdiag(ar1)],
                         axis=1).astype(BF)
    W2e = np.concatenate([W2, W2 @ blkdiag(al2), W2 @ blkdiag(ar2)],
                         axis=1).astype(BF)
    ar2r = np.broadcast_to(ar2.reshape(1, H * C), (P, H * C)).astype(BF).copy()
    b1r = np.broadcast_to(b1.reshape(1, 256), (P, 256)).astype(np.float32).copy()
    b2m = b2.reshape(H, C).mean(axis=0)
    b2r = np.broadcast_to(b2m.reshape(1, C), (P, C)).astype(np.float32).copy()
    add_b1 = bool(np.any(b1)); add_b2 = bool(np.any(b2))

    # ---- per-core edge metadata
    g1 = _g1_row(src0)
    er1 = _er_row(dst0)
    g2 = _g2_row(src1)
    core1 = dst0 // BLK1
    core2 = dst1 // BLK2
    # choose tile counts (max over cores/windows)
    def max_tw(dst, core, blk, nwin):
        mx = 1
        for r in range(NCORES):
            dl = dst[core == r] - r * blk
            cnt = np.bincount(dl // P, minlength=nwin)
            mx = max(mx, int(np.ceil(cnt.max() / P)))
        return mx
    tw1 = max_tw(dst0, core1, BLK1, W1N)
    tw2 = max_tw(dst1, core2, BLK2, W2N)

    er2_idx = _g2_row(np.minimum(np.arange(ER2ROWS), N1 - 1)).astype(np.int32)
    er2_idx = er2_idx.reshape(ER2ROWS // P, P).T.copy()  # [P, chunks]

    in_maps = []
    for r in range(NCORES):
        sel1 = core1 == r
        m1s_, m1e_, m1r_, m1m_ = _pack_edges(
            g1[sel1], dst0[sel1], r * BLK1, BLK1, er1[sel1], tw1)
        sel2 = core2 == r
        m2s_, m2e_, m2r_, m2m_ = _pack_edges(
            g2[sel2], dst1[sel2], r * BLK2, BLK2,
            np.minimum(dst1[sel2], ER2ROWS - 1), tw2)
        rows = np.concatenate([
            np.arange(r * BLK1, (r + 1) * BLK1),
            np.arange(N1 + r * BLK1, N1 + (r + 1) * BLK1)])
        xT_ = np.zeros((F_IN, LP1), BF)
        xT_[:, :LPC1] = x[rows].T.astype(BF)
        in_maps.append(dict(
            xT=xT_, W1e=W1e, W2e=W2e, ar2r=ar2r, b1r=b1r, b2r=b2r,
            m1s=m1s_, m1e=m1e_, m1r=m1r_, m1m=m1m_,
            m2s=m2s_, m2e=m2e_, m2r=m2r_, m2m=m2m_, e2i=er2_idx))

    global _last_in_maps
    _last_in_maps = in_maps
    nc = build_program(tw1, tw2, add_b1, add_b2)
    from concourse.bass_utils import run_bass_kernel_spmd
    res = None
    last_err = None
    for attempt in range(3):
        try:
            res = run_bass_kernel_spmd(nc, in_maps, core_ids=list(range(NCORES)))
            break
        except Exception as e:  # transient device wedge: retry
            last_err = e
            import time as _t
            _t.sleep(10)
    if res is None:
        raise last_err
    out = np.concatenate([res.results[r]["OUT"][:BLK2] for r in range(NCORES)],
                         axis=0)
    return out.astype(np.float32)
